# revision 14
# baseline (speedup 1.0000x reference)
"""Trainium2 Bass kernel for CompGCN(2 layers) + ConvE decoder.

Strategy (8 NeuronCores, SPMD single program, per-core input data):
  * Entities sharded: core c owns rows [6250c, 6250(c+1)).
  * Edge phase: host buckets edges by (side, dest 128-node tile); device
    streams per-edge (col, et, rowloc, nrm); per 128-edge chunk:
    indirect-DMA gathers x[col], r[et]; DVE msg = xg*rg and a scaled
    one-hot (iota==rowloc)*nrm; PE matmul accumulates aggT = msg^T @ onehot
    in PSUM per node tile.  The @W transform is folded after aggregation
    (linearity), BN+tanh applied in transposed layout, then PE transposes +
    AllGather produce the replicated row-major x for the next layer.
  * Decoder: sub/rel indirect gathers; conv via im2col built with one
    indirect DMA from a transposed image (positions sharded per core,
    two passes: stats then fused BN0/BN1+relu into fc); fc K-sharded with
    AllReduce; logits matmul sharded over entities; host concatenates.
"""
import os
import sys
from contextlib import ExitStack

sys.path.insert(0, "/opt/trn_rl_repo")

import numpy as np

import concourse.bass as bass
import concourse.tile as tile
from concourse import bacc, mybir
from concourse.masks import make_identity

F32 = mybir.dt.float32
I32 = mybir.dt.int32
AF = mybir.ActivationFunctionType
OP = mybir.AluOpType

# problem constants (fixed by the reference model)
N_ENT = 50000; N_REL = 200; D0 = 100; D = 200; B = 512; E_DIR = 250000
NF = 200; KER = 7; K2 = 49; OW = 14; NPOS = 196
EPS = 1e-5
NCORES = 8
PSL = N_ENT // NCORES            # 6250 entities per core
NT = (PSL + 127) // 128          # 49 node tiles per core
GB = 4                           # buckets per outT group (512 nodes)
NGROUPS = (NT + GB - 1) // GB    # 13
KG = 1                           # chunks per indirect gather (HW: one row per partition per idma)
SLOTS = 25                       # conv (y,x) slots per core
BT = B // 128                    # 4 batch tiles
NEC = (PSL + 511) // 512         # 13 entity chunks for logits

_CACHE = {}


# --------------------------------------------------------------------------
# host preprocessing
# --------------------------------------------------------------------------

def _graph_prep(edge_index, edge_type):
    ei = np.asarray(edge_index).astype(np.int64)
    ety_all = np.asarray(edge_type).astype(np.int64)
    per = {}
    cpb = np.zeros((2, NT), np.int64)
    for s in range(2):
        row = ei[0, s * E_DIR:(s + 1) * E_DIR]
        col = ei[1, s * E_DIR:(s + 1) * E_DIR]
        ety = ety_all[s * E_DIR:(s + 1) * E_DIR]
        deg = np.bincount(row, minlength=N_ENT).astype(np.float64)
        dinv = np.where(deg > 0, deg ** -0.5, 0.0)
        nrm = (dinv[row] * dinv[col]).astype(np.float32)
        core = row // PSL
        local = row - core * PSL
        bucket = local >> 7
        rowloc = (local & 127).astype(np.float32)
        key = core * NT + bucket
        order = np.argsort(key, kind="stable")
        cnt = np.bincount(key, minlength=NCORES * NT)
        starts = np.zeros(NCORES * NT + 1, np.int64)
        np.cumsum(cnt, out=starts[1:])
        per[s] = (order, starts, col, ety, rowloc, nrm)
        cpb[s] = np.maximum(1, (cnt.reshape(NCORES, NT).max(0) + 127) // 128)
    nch = int(cpb.sum())
    cols_s = np.zeros((NCORES, 128, nch), np.int32)
    ets_s = np.full((NCORES, 128, nch), 400, np.int32)
    rloc_s = np.zeros((NCORES, 128, nch), np.float32)
    nrms_s = np.zeros((NCORES, 128, nch), np.float32)
    ch = 0
    for g in range(NGROUPS):
        for s in range(2):
            order, starts, col, ety, rowloc, nrm = per[s]
            for b in range(g * GB, min(g * GB + GB, NT)):
                n_ch = int(cpb[s][b])
                slots = n_ch * 128
                for c in range(NCORES):
                    k = c * NT + b
                    idx = order[starts[k]:starts[k + 1]]
                    m = len(idx)
                    assert m <= slots, (m, slots)
                    bc = np.zeros(slots, np.int32)
                    be = np.full(slots, 400, np.int32)
                    br = np.zeros(slots, np.float32)
                    bn = np.zeros(slots, np.float32)
                    bc[:m] = col[idx]; be[:m] = ety[idx]
                    br[:m] = rowloc[idx]; bn[:m] = nrm[idx]
                    cols_s[c, :, ch:ch + n_ch] = bc.reshape(n_ch, 128).T
                    ets_s[c, :, ch:ch + n_ch] = be.reshape(n_ch, 128).T
                    rloc_s[c, :, ch:ch + n_ch] = br.reshape(n_ch, 128).T
                    nrms_s[c, :, ch:ch + n_ch] = bn.reshape(n_ch, 128).T
                ch += n_ch
    assert ch == nch
    return cpb, nch, cols_s, ets_s, rloc_s, nrms_s


def _pack_halves(v, parts=100):
    """[parts*k] vector -> [parts, k] column-packed f32."""
    v = np.asarray(v, np.float32).reshape(-1)
    k = v.size // parts
    return v.reshape(k, parts).T.copy()


def _prep_inputs(I):
    """Build per-core input maps (list of dicts) + (cpb, nch)."""
    cpb, nch, cols_s, ets_s, rloc_s, nrms_s = _graph_prep(
        I["edge_index"], I["edge_type"])

    x0 = np.asarray(I["init_embed"], np.float32)
    r1 = np.concatenate(
        [np.asarray(I["init_rel"], np.float32),
         np.asarray(I["loop_rel1"], np.float32)], 0)          # [401,100]
    sub = np.asarray(I["sub"]).astype(np.int64)
    rel = np.asarray(I["rel"]).astype(np.int64)
    sub_idx = sub.reshape(BT, 128).T.astype(np.int32).copy()   # [128, BT]
    rel_idx = rel.reshape(BT, 128).T.astype(np.int32).copy()
    w_conv = np.asarray(I["conv_w"], np.float32).reshape(NF, K2).T.copy()
    fc_w = np.asarray(I["fc_w"], np.float32)                   # [39200, 200]
    ent_bias = np.asarray(I["ent_bias"], np.float32)

    shared = dict(
        x0=x0,
        r1=r1,
        r1T=r1.T.copy(),
        w_in1=np.asarray(I["w_in1"], np.float32),
        w_out1=np.asarray(I["w_out1"], np.float32),
        w_loop1=np.asarray(I["w_loop1"], np.float32),
        w_rel1=np.asarray(I["w_rel1"], np.float32),
        w_in2=np.asarray(I["w_in2"], np.float32),
        w_out2=np.asarray(I["w_out2"], np.float32),
        w_loop2=np.asarray(I["w_loop2"], np.float32),
        w_rel2=np.asarray(I["w_rel2"], np.float32),
        bng1=_pack_halves(I["bn_g1"]), bnb1=_pack_halves(I["bn_b1"]),
        bng2=_pack_halves(I["bn_g2"]), bnb2=_pack_halves(I["bn_b2"]),
        rloop1=_pack_halves(I["loop_rel1"]),                   # [100,1]
        rloop2=_pack_halves(I["loop_rel2"]),                   # [100,2]
        bn0gb=np.stack(
            [np.full(128, np.asarray(I["bn0_g"], np.float32).reshape(-1)[0]),
             np.full(128, np.asarray(I["bn0_b"], np.float32).reshape(-1)[0])],
            1).astype(np.float32),                             # [128,2]
        bn1g=np.concatenate([np.asarray(I["bn1_g"], np.float32),
                             np.zeros(56, np.float32)]).reshape(2, 128).T.copy(),
        bn1b=np.concatenate([np.asarray(I["bn1_b"], np.float32),
                             np.zeros(56, np.float32)]).reshape(2, 128).T.copy(),
        bng2d=_pack_halves(I["bn2_g"]), bnb2d=_pack_halves(I["bn2_b"]),
        w_conv=w_conv,
        sub_idx=sub_idx,
        rel_idx=rel_idx,
    )

    fc_w3 = fc_w.reshape(NF, NPOS, D)
    in_maps = []
    for c in range(NCORES):
        sl = slice(c * PSL, (c + 1) * PSL)
        slots = list(range(c * SLOTS, min(c * SLOTS + SLOTS, NPOS)))
        pidx = np.full((K2, SLOTS), 400, np.int32)
        fcw_core = np.zeros((SLOTS, NF, D), np.float32)
        for si, yx in enumerate(slots):
            y, x = divmod(yx, OW)
            for ky in range(KER):
                for kx in range(KER):
                    pidx[ky * KER + kx, si] = (y + ky) * 20 + (x + kx)
            fcw_core[si] = fc_w3[:, yx, :]
        m = dict(shared)
        m.update(
            x0T=x0[sl].T.copy(),                               # [100, 6250]
            cols=cols_s[c], ets=ets_s[c],
            rloc=rloc_s[c], nrms=nrms_s[c],
            pidx=pidx,
            fcw=fcw_core.reshape(SLOTS * NF, D),
            entb=ent_bias[sl].reshape(1, PSL).copy(),
        )
        in_maps.append(m)
    return cpb, nch, in_maps


# --------------------------------------------------------------------------
# program builder helpers
# --------------------------------------------------------------------------

def _declare_inputs(nc, nch):
    t = {}
    def di(name, shape, dt=F32):
        t[name] = nc.dram_tensor(name, list(shape), dt, kind="ExternalInput")
    di("x0", [N_ENT, D0]); di("r1", [401, D0]); di("r1T", [D0, 401])
    di("x0T", [100, PSL])
    di("cols", [128, nch], I32); di("ets", [128, nch], I32)
    di("rloc", [128, nch]); di("nrms", [128, nch])
    for w in ("w_in1", "w_out1", "w_loop1", "w_rel1"):
        di(w, [D0, D])
    for w in ("w_in2", "w_out2", "w_loop2", "w_rel2"):
        di(w, [D, D])
    di("bng1", [100, 2]); di("bnb1", [100, 2])
    di("bng2", [100, 2]); di("bnb2", [100, 2])
    di("rloop1", [100, 1]); di("rloop2", [100, 2])
    di("bn0gb", [128, 2]); di("bn1g", [128, 2]); di("bn1b", [128, 2])
    di("bng2d", [100, 2]); di("bnb2d", [100, 2])
    di("w_conv", [K2, NF])
    di("sub_idx", [128, BT], I32); di("rel_idx", [128, BT], I32)
    di("pidx", [K2, SLOTS], I32)
    di("fcw", [SLOTS * NF, D])
    di("entb", [1, PSL])
    return t


def _bn_scale_bias(nc, pool, Ssum, Ssq, gcol, bcol, n, pre_scale, name):
    """Stats of u/pre_scale from raw sums of u over n items.
    scale = g / sqrt(var + eps) / pre_scale ; bias = b - mean_u * scale.
    (mean_x = mean_u/pre ; bias = b - mean_x*g*inv = b - mean_u*scale.)"""
    P = Ssum.shape[0]
    mu = pool.tile([P, 1], F32, name=f"{name}_mu")
    nc.vector.tensor_scalar(out=mu[:], in0=Ssum, scalar1=1.0 / n,
                            scalar2=None, op0=OP.mult)
    msq = pool.tile([P, 1], F32, name=f"{name}_msq")
    nc.vector.tensor_tensor(out=msq[:], in0=mu[:], in1=mu[:], op=OP.mult)
    var = pool.tile([P, 1], F32, name=f"{name}_var")
    nc.vector.tensor_scalar(out=var[:], in0=Ssq, scalar1=1.0 / n,
                            scalar2=None, op0=OP.mult)
    nc.vector.tensor_tensor(out=var[:], in0=var[:], in1=msq[:], op=OP.subtract)
    nc.vector.tensor_scalar(out=var[:], in0=var[:],
                            scalar1=1.0 / (pre_scale * pre_scale),
                            scalar2=EPS, op0=OP.mult, op1=OP.add)
    nc.scalar.sqrt(var[:], var[:])
    inv = pool.tile([P, 1], F32, name=f"{name}_inv")
    nc.vector.reciprocal(inv[:], var[:])
    scale = pool.tile([P, 1], F32, name=f"{name}_scale")
    nc.vector.tensor_tensor(out=scale[:], in0=inv[:], in1=gcol, op=OP.mult)
    if pre_scale != 1.0:
        nc.vector.tensor_scalar(out=scale[:], in0=scale[:],
                                scalar1=1.0 / pre_scale, scalar2=None,
                                op0=OP.mult)
    bias = pool.tile([P, 1], F32, name=f"{name}_bias")
    nc.vector.tensor_tensor(out=bias[:], in0=mu[:], in1=scale[:], op=OP.mult)
    nc.vector.tensor_tensor(out=bias[:], in0=bcol, in1=bias[:], op=OP.subtract)
    return scale, bias


def _emit_layer(ctx, tc, consts, keep, Din, x_dram, xT_tiles, r_dram, wts,
                rloop_col, gcol_fn, bcol_fn, sched, streams, lname):
    """One CompGCN layer. Returns [2] tiles [100, PSL] (tanh'd transposed x')."""
    nc = tc.nc
    HN = Din // 100
    iota_f = consts["iota_f"]
    cols_t, ets_t, rloc_t, nrms_t = streams

    lp = ctx.enter_context(tc.tile_pool(name=f"{lname}_work", bufs=3))
    psE = ctx.enter_context(tc.tile_pool(name=f"{lname}_psE", bufs=1, space="PSUM"))
    psO = ctx.enter_context(tc.tile_pool(name=f"{lname}_psO", bufs=1, space="PSUM"))
    agp = ctx.enter_context(tc.tile_pool(name=f"{lname}_agg", bufs=2))

    outT = [keep.tile([100, PSL], F32, name=f"{lname}_outT{h}") for h in range(2)]
    s1 = [lp.tile([100, NGROUPS], F32, tag=f"s1_{h}", bufs=1, name=f"{lname}_s1_{h}")
          for h in range(2)]
    s2 = [lp.tile([100, NGROUPS], F32, tag=f"s2_{h}", bufs=1, name=f"{lname}_s2_{h}")
          for h in range(2)]

    nch = len(sched)
    pse = {}
    msg = None
    for c, (g, s, b, ci, first, last) in enumerate(sched):
        if c % KG == 0:
            kk = min(KG, nch - c)
            xg = lp.tile([128, kk * Din], F32, tag="xg", bufs=3, name="xg")
            nc.gpsimd.indirect_dma_start(
                out=xg[:], out_offset=None, in_=x_dram[:],
                in_offset=bass.IndirectOffsetOnAxis(ap=cols_t[:, c:c + kk], axis=0))
            rg = lp.tile([128, kk * Din], F32, tag="rg", bufs=3, name="rg")
            nc.gpsimd.indirect_dma_start(
                out=rg[:], out_offset=None, in_=r_dram[:],
                in_offset=bass.IndirectOffsetOnAxis(ap=ets_t[:, c:c + kk], axis=0))
            msg = lp.tile([128, kk * Din], F32, tag="msg", bufs=3, name="msg")
            nc.vector.tensor_tensor(out=msg[:], in0=xg[:], in1=rg[:], op=OP.mult)
        j = c % KG
        oh = lp.tile([128, 128], F32, tag="oh", bufs=4, name="oh")
        nc.vector.tensor_scalar(
            out=oh[:], in0=iota_f[:], scalar1=rloc_t[:, c:c + 1],
            scalar2=nrms_t[:, c:c + 1], op0=OP.is_equal, op1=OP.mult)
        if (g, s) not in pse:
            pse[(g, s)] = [psE.tile([100, 512], F32, space="PSUM",
                                    tag=f"e{s}{h}", name=f"pse{s}{h}")
                           for h in range(HN)]
        bl = b - g * GB
        nb = min(128, PSL - 128 * b)
        for h in range(HN):
            nc.tensor.matmul(
                pse[(g, s)][h][:, bl * 128: bl * 128 + nb],
                lhsT=msg[:, j * Din + h * 100: j * Din + h * 100 + 100],
                rhs=oh[:, :nb], start=first, stop=last)

        g_last_b = min(g * GB + GB, NT) - 1
        if s == 1 and b == g_last_b and last:
            # group complete: fold weights, stats
            nn = min(512, PSL - 512 * g)
            aggs = {}
            for sd in range(2):
                aggs[sd] = []
                for h in range(HN):
                    a = agp.tile([100, 512], F32, tag=f"agg{sd}{h}", name="agg")
                    nc.scalar.copy(a[:, :nn], pse[(g, sd)][h][:, :nn])
                    aggs[sd].append(a)
            del pse[(g, 0)], pse[(g, 1)]
            loopT = []
            for kh in range(HN):
                lt = lp.tile([100, 512], F32, tag=f"lt{kh}", name="lt")
                nc.vector.tensor_scalar(
                    out=lt[:, :nn], in0=xT_tiles[kh][:, g * 512: g * 512 + nn],
                    scalar1=rloop_col[:, kh:kh + 1], scalar2=None, op0=OP.mult)
                loopT.append(lt)
            w_in, w_out, w_loop = wts
            for h2 in range(2):
                po = psO.tile([100, 512], F32, space="PSUM", tag=f"o{h2}",
                              name="po")
                n_mm = 3 * HN
                i_mm = 0
                for (wt, rhs_list) in ((w_in, aggs[0]), (w_out, aggs[1]),
                                       (w_loop, loopT)):
                    for kh in range(HN):
                        nc.tensor.matmul(
                            po[:, :nn],
                            lhsT=wt[kh][:, h2 * 100: h2 * 100 + 100],
                            rhs=rhs_list[kh][:, :nn],
                            start=(i_mm == 0), stop=(i_mm == n_mm - 1))
                        i_mm += 1
                nc.scalar.activation(
                    out=outT[h2][:, g * 512: g * 512 + nn], in_=po[:, :nn],
                    func=AF.Copy, accum_out=s1[h2][:, g:g + 1])
                scr = lp.tile([100, 512], F32, tag="scr", name="scr")
                nc.scalar.activation(
                    out=scr[:, :nn], in_=po[:, :nn],
                    func=AF.Square, accum_out=s2[h2][:, g:g + 1])

    # BN stats allreduce
    bnred = lp.tile([100, 4], F32, tag="bnred", bufs=1, name="bnred")
    for h2 in range(2):
        nc.vector.tensor_reduce(out=bnred[:, h2:h2 + 1], in_=s1[h2][:],
                                axis=mybir.AxisListType.X, op=OP.add)
        nc.vector.tensor_reduce(out=bnred[:, 2 + h2:3 + h2], in_=s2[h2][:],
                                axis=mybir.AxisListType.X, op=OP.add)
    bnin = nc.dram_tensor(f"{lname}_bnin", [100, 4], F32)
    bnout = nc.dram_tensor(f"{lname}_bnout", [100, 4], F32, addr_space="Shared")
    nc.gpsimd.dma_start(bnin[:], bnred[:])
    nc.gpsimd.collective_compute(
        "AllReduce", OP.add, replica_groups=[list(range(NCORES))],
        ins=[bnin[:]], outs=[bnout[:]])
    bnsum = lp.tile([100, 4], F32, tag="bnsum", bufs=1, name="bnsum")
    nc.sync.dma_start(bnsum[:], bnout[:])

    sp = ctx.enter_context(tc.tile_pool(name=f"{lname}_bn", bufs=1))
    for h2 in range(2):
        scale, bias = _bn_scale_bias(
            nc, sp, bnsum[:, h2:h2 + 1], bnsum[:, 2 + h2:3 + h2],
            gcol_fn(h2), bcol_fn(h2), float(N_ENT), 3.0, f"{lname}bn{h2}")
        for g in range(NGROUPS):
            nn = min(512, PSL - 512 * g)
            nc.scalar.activation(
                out=outT[h2][:, g * 512: g * 512 + nn],
                in_=outT[h2][:, g * 512: g * 512 + nn],
                func=AF.Tanh, bias=bias[:], scale=scale[:])
    return outT


def _emit_transpose_allgather(ctx, tc, consts, xT, bounce, full, lname):
    nc = tc.nc
    ident = consts["ident"]
    tp = ctx.enter_context(tc.tile_pool(name=f"{lname}_tr", bufs=3))
    psT = ctx.enter_context(tc.tile_pool(name=f"{lname}_psT", bufs=2, space="PSUM"))
    for t in range(NT):
        nb = min(128, PSL - 128 * t)
        xrow = tp.tile([128, D], F32, tag="xrow", name="xrow")
        for h2 in range(2):
            pt = psT.tile([128, 128], F32, space="PSUM", tag="pt", name="pt")
            nc.tensor.transpose(pt[:nb, :100],
                                xT[h2][:, t * 128: t * 128 + nb],
                                ident[:100, :100])
            nc.scalar.copy(xrow[:nb, h2 * 100: h2 * 100 + 100], pt[:nb, :100])
        nc.sync.dma_start(bounce[t * 128: t * 128 + nb, :], xrow[:nb, :])
    nc.gpsimd.collective_compute(
        "AllGather", OP.bypass, replica_groups=[list(range(NCORES))],
        ins=[bounce[:]], outs=[full[:]])


def _emit_rel_update(ctx, tc, consts, keep, rT_tiles, w_rel, out_dram,
                     loop_col, lname):
    """rT' = w_rel^T @ rT (per Dout half); writes row-major [401, D] DRAM.
    Returns [2] tiles [100, 401]."""
    nc = tc.nc
    ident = consts["ident"]
    HN = len(rT_tiles)
    rp = ctx.enter_context(tc.tile_pool(name=f"{lname}_r", bufs=2))
    psR = ctx.enter_context(tc.tile_pool(name=f"{lname}_psR", bufs=2, space="PSUM"))
    newT = []
    for h2 in range(2):
        pr = psR.tile([100, 512], F32, space="PSUM", tag="pr", name="pr")
        for kh in range(HN):
            nc.tensor.matmul(
                pr[:, :401],
                lhsT=w_rel[kh][:, h2 * 100: h2 * 100 + 100],
                rhs=rT_tiles[kh][:, :401],
                start=(kh == 0), stop=(kh == HN - 1))
        rt = keep.tile([100, 401], F32, name=f"{lname}_rT{h2}")
        nc.scalar.copy(rt[:, :], pr[:, :401])
        if loop_col is not None:
            nc.vector.tensor_copy(rt[:, 400:401], loop_col[:, h2:h2 + 1])
        newT.append(rt)
    for q in range(4):
        nb = min(128, 401 - q * 128)
        rrow = rp.tile([128, D], F32, tag="rrow", name="rrow")
        for h2 in range(2):
            pt = psR.tile([128, 128], F32, space="PSUM", tag="pt", name="pt")
            nc.tensor.transpose(pt[:nb, :100],
                                newT[h2][:, q * 128: q * 128 + nb],
                                ident[:100, :100])
            nc.scalar.copy(rrow[:nb, h2 * 100: h2 * 100 + 100], pt[:nb, :100])
        nc.sync.dma_start(out_dram[q * 128: q * 128 + nb, :], rrow[:nb, :])
    return newT


def _emit_decoder(ctx, tc, consts, tin, keep, x2T, x2_full, r3_dram, out_slice):
    nc = tc.nc
    ident, ones100, ones1 = consts["ident"], consts["ones100"], consts["ones1"]
    dp = ctx.enter_context(tc.tile_pool(name="dec", bufs=2))
    for nm, P in (("bn0gb", 128), ("bn1g", 128), ("bn1b", 128),
                  ("bng2d", 100), ("bnb2d", 100)):
        t = dp.tile([P, 2], F32, tag=nm, bufs=1, name=nm + "_t")
        nc.sync.dma_start(t[:], tin[nm][:])
        tin[nm + "_t"] = t

    subT = [keep.tile([100, B], F32, name=f"subT{h}") for h in range(2)]
    relT = [keep.tile([100, B], F32, name=f"relT{h}") for h in range(2)]
    imgT = nc.dram_tensor("imgT", [401, B], F32)

    with ExitStack() as sa:
        psA = sa.enter_context(tc.tile_pool(name="dec_psA", bufs=2, space="PSUM"))
        sub_t = dp.tile([128, BT], I32, tag="subi", bufs=1, name="sub_t")
        nc.sync.dma_start(sub_t[:], tin["sub_idx"][:])
        rel_t = dp.tile([128, BT], I32, tag="reli", bufs=1, name="rel_t")
        nc.sync.dma_start(rel_t[:], tin["rel_idx"][:])
        se = dp.tile([128, BT * D], F32, tag="se", bufs=1, name="se")
        re = dp.tile([128, BT * D], F32, tag="re", bufs=1, name="re")
        for j in range(BT):
            nc.gpsimd.indirect_dma_start(
                out=se[:, j * D:(j + 1) * D], out_offset=None, in_=x2_full[:],
                in_offset=bass.IndirectOffsetOnAxis(ap=sub_t[:, j:j + 1], axis=0))
            nc.gpsimd.indirect_dma_start(
                out=re[:, j * D:(j + 1) * D], out_offset=None, in_=r3_dram[:],
                in_offset=bass.IndirectOffsetOnAxis(ap=rel_t[:, j:j + 1], axis=0))
        for src, dstl in ((se, subT), (re, relT)):
            for j in range(BT):
                for h2 in range(2):
                    pt = psA.tile([128, 128], F32, space="PSUM", tag="pt",
                                  name="pt")
                    nc.tensor.transpose(
                        pt[:100, :],
                        src[:, j * D + h2 * 100: j * D + h2 * 100 + 100],
                        ident[:, :])
                    nc.scalar.copy(dstl[h2][:, j * 128: j * 128 + 128],
                                   pt[:100, :])

        # BN0 scalar stats (full batch, replicated on every core)
        p1 = dp.tile([100, 4], F32, tag="p1", bufs=1, name="p1")
        p2 = dp.tile([100, 4], F32, tag="p2", bufs=1, name="p2")
        scr = dp.tile([100, B], F32, tag="dscr", bufs=1, name="dscr")
        for i, t in enumerate((subT[0], subT[1], relT[0], relT[1])):
            nc.vector.tensor_reduce(out=p1[:, i:i + 1], in_=t[:],
                                    axis=mybir.AxisListType.X, op=OP.add)
            nc.scalar.activation(out=scr[:], in_=t[:], func=AF.Square,
                                 accum_out=p2[:, i:i + 1])
        pk = dp.tile([100, 2], F32, tag="pk", bufs=1, name="pk")
        nc.vector.tensor_reduce(out=pk[:, 0:1], in_=p1[:],
                                axis=mybir.AxisListType.X, op=OP.add)
        nc.vector.tensor_reduce(out=pk[:, 1:2], in_=p2[:],
                                axis=mybir.AxisListType.X, op=OP.add)
        ps_sc = psA.tile([1, 2], F32, space="PSUM", tag="sc", name="ps_sc")
        nc.tensor.matmul(ps_sc[:, :], lhsT=ones100[:, :], rhs=pk[:, :],
                         start=True, stop=True)
        sc_row = dp.tile([1, 2], F32, tag="scrow", bufs=1, name="sc_row")
        nc.scalar.copy(sc_row[:, :], ps_sc[:, :])
        ps_bc = psA.tile([128, 2], F32, space="PSUM", tag="bc", name="ps_bc")
        nc.tensor.matmul(ps_bc[:, :], lhsT=ones1[:, :], rhs=sc_row[:, :],
                         start=True, stop=True)
        bc = keep.tile([128, 2], F32, name="bc")
        nc.scalar.copy(bc[:, :], ps_bc[:, :])

        # imgT rows p=2d+c: even rows sub, odd rel; row 400 zero
        for h2 in range(2):
            nc.sync.dma_start(imgT[h2 * 200: h2 * 200 + 200: 2, :],
                              subT[h2][:, :])
            nc.sync.dma_start(imgT[h2 * 200 + 1: h2 * 200 + 200: 2, :],
                              relT[h2][:, :])
        zrow = dp.tile([1, B], F32, tag="zrow", bufs=1, name="zrow")
        nc.vector.memset(zrow[:], 0.0)
        nc.sync.dma_start(imgT[400:401, :], zrow[:, :])

    bp = ctx.enter_context(tc.tile_pool(name="dec_bn0", bufs=1))
    bn0gb = tin["bn0gb_t"]
    s0f, _t0f = _bn_scale_bias(nc, bp, bc[:, 0:1], bc[:, 1:2],
                               bn0gb[:, 0:1], bn0gb[:, 1:2],
                               float(B * 400), 1.0, "bn0")

    wconv_t = keep.tile([K2, NF], F32, name="wconv_t")
    nc.sync.dma_start(wconv_t[:], tin["w_conv"][:])
    pidx_t = keep.tile([K2, SLOTS], I32, name="pidx_t")
    nc.sync.dma_start(pidx_t[:], tin["pidx"][:])

    # ---- conv pass A: stats only ----
    st1 = [keep.tile([128, SLOTS], F32, name=f"st1_{a}") for a in range(2)]
    st2 = [keep.tile([128, SLOTS], F32, name=f"st2_{a}") for a in range(2)]
    PCH = 5
    FH = ((0, 128), (128, 72))
    with ExitStack() as sb:
        psB = sb.enter_context(tc.tile_pool(name="dec_psB", bufs=2, space="PSUM"))
        cp_ = sb.enter_context(tc.tile_pool(name="dec_convA", bufs=2))
        cscr = cp_.tile([128, B], F32, tag="cscr", bufs=1, name="cscr")
        for pc in range(SLOTS // PCH):
            Pt = cp_.tile([K2, PCH * B], F32, tag="P", name="Pt")
            for sl_ in range(PCH):
                nc.gpsimd.indirect_dma_start(
                    out=Pt[:, sl_ * B:(sl_ + 1) * B], out_offset=None, in_=imgT[:],
                    in_offset=bass.IndirectOffsetOnAxis(
                        ap=pidx_t[:, pc * PCH + sl_: pc * PCH + sl_ + 1], axis=0))
            for sl in range(PCH):
                gi = pc * PCH + sl
                for a, (f0, fn_) in enumerate(FH):
                    psc = psB.tile([128, B], F32, space="PSUM", tag=f"c{a}",
                                   name="psc")
                    nc.tensor.matmul(psc[:fn_, :],
                                     lhsT=wconv_t[:, f0:f0 + fn_],
                                     rhs=Pt[:, sl * B:(sl + 1) * B],
                                     start=True, stop=True)
                    nc.scalar.activation(
                        out=cscr[:fn_, :], in_=psc[:fn_, :],
                        func=AF.Copy, accum_out=st1[a][:fn_, gi:gi + 1])
                    nc.scalar.activation(
                        out=cscr[:fn_, :], in_=psc[:fn_, :],
                        func=AF.Square, accum_out=st2[a][:fn_, gi:gi + 1])

    # ---- BN1 stats allreduce + fold with BN0 ----
    bn1red = dp.tile([128, 4], F32, tag="bn1red", bufs=1, name="bn1red")
    nc.vector.memset(bn1red[:], 0.0)
    for a, (f0, fn_) in enumerate(FH):
        nc.vector.tensor_reduce(out=bn1red[:fn_, a * 2:a * 2 + 1],
                                in_=st1[a][:fn_, :],
                                axis=mybir.AxisListType.X, op=OP.add)
        nc.vector.tensor_reduce(out=bn1red[:fn_, a * 2 + 1:a * 2 + 2],
                                in_=st2[a][:fn_, :],
                                axis=mybir.AxisListType.X, op=OP.add)
    b1in = nc.dram_tensor("b1in", [128, 4], F32)
    b1out = nc.dram_tensor("b1out", [128, 4], F32, addr_space="Shared")
    nc.gpsimd.dma_start(b1in[:], bn1red[:])
    nc.gpsimd.collective_compute(
        "AllReduce", OP.add, replica_groups=[list(range(NCORES))],
        ins=[b1in[:]], outs=[b1out[:]])
    bn1sum = dp.tile([128, 4], F32, tag="bn1sum", bufs=1, name="bn1sum")
    nc.sync.dma_start(bn1sum[:], b1out[:])

    # relu input on RAW conv values: scale_h = s0*inv1*g1,
    # bias_h = b1 - s0*mc*inv1*g1  (t0*sumw cancels against m1 exactly)
    cntc = float(B * NPOS)
    fb = ctx.enter_context(tc.tile_pool(name="dec_bn1", bufs=1))
    scale_h, bias_h = [], []
    for a in range(2):
        mc = fb.tile([128, 1], F32, name=f"mc{a}")
        nc.vector.tensor_scalar(out=mc[:], in0=bn1sum[:, a * 2:a * 2 + 1],
                                scalar1=1.0 / cntc, scalar2=None, op0=OP.mult)
        vc = fb.tile([128, 1], F32, name=f"vc{a}")
        nc.vector.tensor_scalar(out=vc[:], in0=bn1sum[:, a * 2 + 1:a * 2 + 2],
                                scalar1=1.0 / cntc, scalar2=None, op0=OP.mult)
        msq = fb.tile([128, 1], F32, name=f"msq{a}")
        nc.vector.tensor_tensor(out=msq[:], in0=mc[:], in1=mc[:], op=OP.mult)
        nc.vector.tensor_tensor(out=vc[:], in0=vc[:], in1=msq[:], op=OP.subtract)
        s0sq = fb.tile([128, 1], F32, name=f"s0sq{a}")
        nc.vector.tensor_tensor(out=s0sq[:], in0=s0f[:], in1=s0f[:], op=OP.mult)
        nc.vector.tensor_tensor(out=vc[:], in0=vc[:], in1=s0sq[:], op=OP.mult)
        nc.vector.tensor_scalar(out=vc[:], in0=vc[:], scalar1=1.0, scalar2=EPS,
                                op0=OP.mult, op1=OP.add)
        nc.scalar.sqrt(vc[:], vc[:])
        inv1 = fb.tile([128, 1], F32, name=f"inv1{a}")
        nc.vector.reciprocal(inv1[:], vc[:])
        ig = fb.tile([128, 1], F32, name=f"ig{a}")
        nc.vector.tensor_tensor(out=ig[:], in0=inv1[:],
                                in1=tin["bn1g_t"][:, a:a + 1], op=OP.mult)
        sh = fb.tile([128, 1], F32, name=f"sh{a}")
        nc.vector.tensor_tensor(out=sh[:], in0=s0f[:], in1=ig[:], op=OP.mult)
        bh = fb.tile([128, 1], F32, name=f"bh{a}")
        nc.vector.tensor_tensor(out=bh[:], in0=s0f[:], in1=mc[:], op=OP.mult)
        nc.vector.tensor_tensor(out=bh[:], in0=bh[:], in1=ig[:], op=OP.mult)
        nc.vector.tensor_tensor(out=bh[:], in0=tin["bn1b_t"][:, a:a + 1],
                                in1=bh[:], op=OP.subtract)
        scale_h.append(sh); bias_h.append(bh)

    # ---- conv pass B + fc (K-sharded over this core's slots) ----
    fcin = nc.dram_tensor("fcin", [B, NF], F32)
    fcout = nc.dram_tensor("fcout", [B, NF], F32, addr_space="Shared")
    with ExitStack() as sc_:
        psC = sc_.enter_context(tc.tile_pool(name="dec_psC", bufs=1, space="PSUM"))
        fcp = sc_.enter_context(tc.tile_pool(name="dec_fc", bufs=3))
        psF = [psC.tile([128, NF], F32, space="PSUM", tag=f"f{bc}", bufs=1,
                        name=f"psF{bc}") for bc in range(BT)]
        n_kt = SLOTS * 2
        ik = 0
        for pc in range(SLOTS // PCH):
            Pt = fcp.tile([K2, PCH * B], F32, tag="P2", bufs=2, name="Pt2")
            for sl_ in range(PCH):
                nc.gpsimd.indirect_dma_start(
                    out=Pt[:, sl_ * B:(sl_ + 1) * B], out_offset=None, in_=imgT[:],
                    in_offset=bass.IndirectOffsetOnAxis(
                        ap=pidx_t[:, pc * PCH + sl_: pc * PCH + sl_ + 1], axis=0))
            for sl in range(PCH):
                gi = pc * PCH + sl
                for a, (f0, fn_) in enumerate(FH):
                    psc = psC.tile([128, B], F32, space="PSUM", tag=f"c{a}",
                                   bufs=2, name="psc2")
                    nc.tensor.matmul(psc[:fn_, :],
                                     lhsT=wconv_t[:, f0:f0 + fn_],
                                     rhs=Pt[:, sl * B:(sl + 1) * B],
                                     start=True, stop=True)
                    stg = fcp.tile([128, B], F32, tag="stg", name="stg")
                    nc.scalar.activation(out=stg[:fn_, :], in_=psc[:fn_, :],
                                         func=AF.Relu, bias=bias_h[a][:fn_],
                                         scale=scale_h[a][:fn_])
                    fcw_t = fcp.tile([128, D], F32, tag="fcw", name="fcw_t")
                    nc.sync.dma_start(
                        fcw_t[:fn_, :],
                        tin["fcw"][gi * NF + f0: gi * NF + f0 + fn_, :])
                    for bc2 in range(BT):
                        nc.tensor.matmul(
                            psF[bc2][:, :],
                            lhsT=stg[:fn_, bc2 * 128:(bc2 + 1) * 128],
                            rhs=fcw_t[:fn_, :],
                            start=(ik == 0), stop=(ik == n_kt - 1))
                    ik += 1
        for bc2 in range(BT):
            fsb = fcp.tile([128, NF], F32, tag="fsb", name="fsb")
            nc.scalar.copy(fsb[:, :], psF[bc2][:, :])
            nc.sync.dma_start(fcin[bc2 * 128:(bc2 + 1) * 128, :], fsb[:, :])
    nc.gpsimd.collective_compute(
        "AllReduce", OP.add, replica_groups=[list(range(NCORES))],
        ins=[fcin[:]], outs=[fcout[:]])

    # ---- BN2 + relu in transposed layout ----
    h2T = [keep.tile([100, B], F32, name=f"h2T{h}") for h in range(2)]
    with ExitStack() as sd:
        psD2 = sd.enter_context(tc.tile_pool(name="dec_psD2", bufs=2, space="PSUM"))
        fp2 = sd.enter_context(tc.tile_pool(name="dec_f2", bufs=2))
        for bc2 in range(BT):
            fr = fp2.tile([128, NF], F32, tag="fr", name="fr")
            nc.sync.dma_start(fr[:, :], fcout[bc2 * 128:(bc2 + 1) * 128, :])
            for h2 in range(2):
                pt = psD2.tile([128, 128], F32, space="PSUM", tag="pt", name="pt")
                nc.tensor.transpose(pt[:100, :],
                                    fr[:, h2 * 100: h2 * 100 + 100], ident[:, :])
                nc.scalar.copy(h2T[h2][:, bc2 * 128:(bc2 + 1) * 128], pt[:100, :])
        b2p = sd.enter_context(tc.tile_pool(name="dec_bn2", bufs=1))
        dscr2 = fp2.tile([100, B], F32, tag="dscr2", bufs=1, name="dscr2")
        for h2 in range(2):
            sS = b2p.tile([100, 1], F32, name=f"fs1_{h2}")
            nc.vector.tensor_reduce(out=sS[:], in_=h2T[h2][:],
                                    axis=mybir.AxisListType.X, op=OP.add)
            sQ = b2p.tile([100, 1], F32, name=f"fs2_{h2}")
            nc.scalar.activation(out=dscr2[:], in_=h2T[h2][:], func=AF.Square,
                                 accum_out=sQ[:])
            sc2, bi2 = _bn_scale_bias(nc, b2p, sS[:], sQ[:],
                                      tin["bng2d_t"][:, h2:h2 + 1],
                                      tin["bnb2d_t"][:, h2:h2 + 1],
                                      float(B), 1.0, f"bn2_{h2}")
            nc.scalar.activation(out=h2T[h2][:], in_=h2T[h2][:], func=AF.Relu,
                                 bias=bi2[:], scale=sc2[:])

    # ---- logits [512, PSL] slice ----
    with ExitStack() as se_:
        psL = se_.enter_context(tc.tile_pool(name="dec_psL", bufs=3, space="PSUM"))
        lgp = se_.enter_context(tc.tile_pool(name="dec_lg", bufs=3))
        entb_t = lgp.tile([1, PSL], F32, tag="entb", bufs=1, name="entb_t")
        nc.sync.dma_start(entb_t[:], tin["entb"][:])
        for bc2 in range(BT):
            for ec in range(NEC):
                nn = min(512, PSL - ec * 512)
                pl = psL.tile([128, 512], F32, space="PSUM", tag="pl", name="pl")
                nc.tensor.matmul(pl[:, :nn],
                                 lhsT=h2T[0][:, bc2 * 128:(bc2 + 1) * 128],
                                 rhs=x2T[0][:, ec * 512: ec * 512 + nn],
                                 start=True, stop=False)
                nc.tensor.matmul(pl[:, :nn],
                                 lhsT=h2T[1][:, bc2 * 128:(bc2 + 1) * 128],
                                 rhs=x2T[1][:, ec * 512: ec * 512 + nn],
                                 start=False, stop=False)
                nc.tensor.matmul(pl[:, :nn], lhsT=ones1[:, :],
                                 rhs=entb_t[0:1, ec * 512: ec * 512 + nn],
                                 start=False, stop=True)
                sig = lgp.tile([128, 512], F32, tag="sig", name="sig")
                nc.scalar.activation(out=sig[:, :nn], in_=pl[:, :nn],
                                     func=AF.Sigmoid)
                nc.sync.dma_start(
                    out_slice[bc2 * 128:(bc2 + 1) * 128,
                              ec * 512: ec * 512 + nn],
                    sig[:, :nn])


def _build_program(cpb, nch):
    nc = bacc.Bacc("TRN2", target_bir_lowering=False, debug=False)
    tin = _declare_inputs(nc, nch)
    out_slice = nc.dram_tensor("out_slice", [B, PSL], F32, kind="ExternalOutput")

    sched = []
    for g in range(NGROUPS):
        for s in range(2):
            for b in range(g * GB, min(g * GB + GB, NT)):
                n_ch = int(cpb[s][b])
                for ci in range(n_ch):
                    sched.append((g, s, b, ci, ci == 0, ci == n_ch - 1))
    assert len(sched) == nch

    x1_bounce = nc.dram_tensor("x1_bounce", [PSL, D], F32)
    x1_full = nc.dram_tensor("x1_full", [N_ENT, D], F32, addr_space="Shared")
    x2_bounce = nc.dram_tensor("x2_bounce", [PSL, D], F32)
    x2_full = nc.dram_tensor("x2_full", [N_ENT, D], F32, addr_space="Shared")
    r2_dram = nc.dram_tensor("r2_dram", [401, D], F32)
    r3_dram = nc.dram_tensor("r3_dram", [401, D], F32)

    with tile.TileContext(nc) as tc, ExitStack() as top:
        cp = top.enter_context(tc.tile_pool(name="const", bufs=1))
        ident = cp.tile([128, 128], F32, name="ident")
        make_identity(nc, ident[:])
        iota_i = cp.tile([128, 128], I32, name="iota_i")
        nc.gpsimd.iota(iota_i[:], pattern=[[1, 128]], base=0, channel_multiplier=0)
        iota_f = cp.tile([128, 128], F32, name="iota_f")
        nc.vector.tensor_copy(iota_f[:], iota_i[:])
        ones100 = cp.tile([100, 1], F32, name="ones100")
        nc.vector.memset(ones100[:], 1.0)
        ones1 = cp.tile([1, 128], F32, name="ones1")
        nc.vector.memset(ones1[:], 1.0)
        consts = dict(ident=ident, iota_f=iota_f, ones100=ones100, ones1=ones1)

        keep = top.enter_context(tc.tile_pool(name="keep", bufs=1))

        with ExitStack() as ph:
            pk = ph.enter_context(tc.tile_pool(name="keep12", bufs=1))

            def load(name, shape, dt=F32, pool=pk):
                t = pool.tile(list(shape), dt, name=f"{name}_t")
                nc.sync.dma_start(t[:], tin[name][:])
                return t

            def load_w(name, Din):
                ts = []
                for kh in range(Din // 100):
                    t = pk.tile([100, D], F32, name=f"{name}_t{kh}")
                    nc.sync.dma_start(t[:], tin[name][kh * 100:(kh + 1) * 100, :])
                    ts.append(t)
                return ts

            cols_t = load("cols", [128, nch], I32)
            ets_t = load("ets", [128, nch], I32)
            rloc_t = load("rloc", [128, nch])
            nrms_t = load("nrms", [128, nch])
            streams = (cols_t, ets_t, rloc_t, nrms_t)
            w1 = {w: load_w(w, D0)
                  for w in ("w_in1", "w_out1", "w_loop1", "w_rel1")}
            w2 = {w: load_w(w, D)
                  for w in ("w_in2", "w_out2", "w_loop2", "w_rel2")}
            bng1 = load("bng1", [100, 2]); bnb1 = load("bnb1", [100, 2])
            bng2 = load("bng2", [100, 2]); bnb2 = load("bnb2", [100, 2])
            rloop1 = load("rloop1", [100, 1]); rloop2 = load("rloop2", [100, 2])
            r1T_t = load("r1T", [D0, 401])

            with ExitStack() as l1ctx:
                pk0 = l1ctx.enter_context(tc.tile_pool(name="keep0", bufs=1))
                x0T_t = load("x0T", [100, PSL], pool=pk0)
                x1T = _emit_layer(
                    l1ctx, tc, consts, pk, D0, tin["x0"], [x0T_t], tin["r1"],
                    (w1["w_in1"], w1["w_out1"], w1["w_loop1"]), rloop1,
                    lambda h2: bng1[:, h2:h2 + 1], lambda h2: bnb1[:, h2:h2 + 1],
                    sched, streams, "L1")
            with ExitStack() as tctx:
                _emit_transpose_allgather(tctx, tc, consts, x1T, x1_bounce,
                                          x1_full, "L1")
                r2T = _emit_rel_update(tctx, tc, consts, pk, [r1T_t],
                                       w1["w_rel1"], r2_dram, rloop2, "R2")

            with ExitStack() as l2ctx:
                x2T = _emit_layer(
                    l2ctx, tc, consts, keep, D, x1_full, x1T, r2_dram,
                    (w2["w_in2"], w2["w_out2"], w2["w_loop2"]), rloop2,
                    lambda h2: bng2[:, h2:h2 + 1], lambda h2: bnb2[:, h2:h2 + 1],
                    sched, streams, "L2")
            with ExitStack() as tctx:
                _emit_transpose_allgather(tctx, tc, consts, x2T, x2_bounce,
                                          x2_full, "L2")
                rloc_p = tctx.enter_context(tc.tile_pool(name="r3loc", bufs=1))
                _emit_rel_update(tctx, tc, consts, rloc_p, r2T,
                                 w2["w_rel2"], r3_dram, None, "R3")

        with ExitStack() as dctx:
            _emit_decoder(dctx, tc, consts, tin, keep, x2T, x2_full, r3_dram,
                          out_slice)

        dbg = os.environ.get("KDEBUG", "")
        for name, src, shape in (("dbg_x1", x1_bounce, [PSL, D]),
                                 ("dbg_x2", x2_bounce, [PSL, D]),
                                 ("dbg_r2", r2_dram, [401, D]),
                                 ("dbg_r3", r3_dram, [401, D])):
            if name[4:] in dbg:
                t = nc.dram_tensor(name, shape, F32, kind="ExternalOutput")
                nc.gpsimd.dma_start(t[:], src[:])

    nc.finalize()
    return nc


# --------------------------------------------------------------------------
# entry point
# --------------------------------------------------------------------------

def _get_program(cpb, nch):
    key = (tuple(cpb.reshape(-1).tolist()), nch)
    if key not in _CACHE:
        _CACHE[key] = _build_program(cpb, nch)
    return _CACHE[key]


def kernel(**inputs) -> np.ndarray:
    from concourse.bass_utils import run_bass_kernel_spmd
    cpb, nch, in_maps = _prep_inputs(inputs)
    nc = _get_program(cpb, nch)
    res = run_bass_kernel_spmd(nc, in_maps, core_ids=list(range(NCORES)))
    out = np.concatenate([res.results[c]["out_slice"] for c in range(NCORES)],
                         axis=1)
    return out.astype(np.float32)
